# revision 26
# baseline (speedup 1.0000x reference)
"""FPN ROIAlign pooler (nn_Pooler) on 8 trn2 cores — TensorEngine version.

Strategy: data-parallel over RoIs. Host builds a channels-last fp16 pixel
table [161500px, 256ch] and, per box, the distinct 4-pixel-aligned UNITS
(flat blocks of the table, 2KB each) its 7x7x(2x2) bilinear sampling grid
touches, plus a sparse weight matrix W[slot, 4, 49bins] (fp16). Device:
batched dma_gather (prepare_only + trigger, alternating 2 SWDGE queues)
pulls unit rows into SBUF tiles [128 units, 1024ch]; TensorE accumulates
P[49bins, 256ch] += W[:,t,q,:].T @ F[:,t,q*256:(q+1)*256] over the box's
tiles in PSUM (q = pixel offset in unit); scalar engine copies PSUM->SBUF
bf16; DMA out. The vector engine is idle by design (v1 bottleneck, 87%
busy); 4-px units quarter the SWDGE descriptor count vs per-pair gathers
(the v2 bottleneck: gpsimd descriptor generation at ~8-10ns/row).

Boxes are dealt to cores in rounds of 8 (one box per core per round),
sorted per level-group by descending unit count, so every core executes an
identical static instruction stream (SPMD) with per-core data.
"""
import numpy as np
from contextlib import ExitStack

from concourse import bacc, bass, mybir, tile, bass_utils

C = 256
N_CORES = 8
OUT = 7
NBIN = OUT * OUT
LVL_HW = [(200, 304), (100, 152), (50, 76), (25, 38)]
SCALES = (0.25, 0.125, 0.0625, 0.03125)
SEG_SZ = [h * w for h, w in LVL_HW]           # px per (lvl, batch) segment
# segment order: (0,0),(0,1),(1,0),(1,1),(2,0),(2,1),(3,0),(3,1)
SEG_BASE = np.zeros((4, 2), np.int64)
_off = 0
for _l in range(4):
    for _b in range(2):
        SEG_BASE[_l, _b] = _off
        _off += SEG_SZ[_l]
TOTAL_PX = int(_off)                           # 161500
END_PAD_PX = 4
TABLE_PX = TOTAL_PX + END_PAD_PX

UNIT_PX = 4                       # pixels per gather unit (flat block)
UNIT_ELEM = UNIT_PX * C           # fp16 elements per unit row (2KB)
# gather groups: (base_px, n_units). Units are flat UNIT_PX blocks.
PGROUPS = [
    (0, 15200),        # lvl0 batch0
    (60800, 15200),    # lvl0 batch1
    (121600, 7600),    # lvl1 both batches
    (152000, 2375),    # lvl2+lvl3 all (last unit ends exactly at 161500)
]
THBATCH = 12                      # max 64-unit half-tiles per dma_gather call
NQUEUE = 2                        # SWDGE queues, alternated per batch

_nc_cache = {}


def _build_nc(sig):
    """sig: tuple of batches; each batch = (group, (th_round0, th_round1, ...))
    where th = half-tile (64-unit) count of the round. Rounds pack densely at
    half-column granularity inside a batch's gather stream."""
    nc = bacc.Bacc("TRN2", target_bir_lowering=False, debug=False,
                   num_devices=N_CORES, dynamic_dma_scratch_size=65536,
                   num_swdge_queues=NQUEUE)
    rounds_total = sum(len(ths) for _, ths in sig)
    cols_total = sum((sum(ths) + 1) // 2 for _, ths in sig)
    idx_cols = sum(sum(ths) * 4 for _, ths in sig)
    w_cols = cols_total * UNIT_PX * NBIN

    table_d = nc.dram_tensor("table", [TABLE_PX * C], mybir.dt.float16,
                             kind="ExternalInput")
    idx_d = nc.dram_tensor("idxs", [128, idx_cols], mybir.dt.int16,
                           kind="ExternalInput")
    w_d = nc.dram_tensor("wts", [128, w_cols], mybir.dt.float16,
                         kind="ExternalInput")
    out_d = nc.dram_tensor("out", [rounds_total * NBIN, C], mybir.dt.bfloat16,
                           kind="ExternalOutput")

    with tile.TileContext(nc) as tc, ExitStack() as ctx:
        sbi = ctx.enter_context(tc.tile_pool(name="sbi", bufs=1))
        sbf = ctx.enter_context(tc.tile_pool(name="sbf", bufs=3))
        sbw = ctx.enter_context(tc.tile_pool(name="sbw", bufs=3))
        sbo = ctx.enter_context(tc.tile_pool(name="sbo", bufs=3))
        psm = ctx.enter_context(tc.tile_pool(name="psm", bufs=4, space="PSUM"))

        idx_t = sbi.tile([128, idx_cols], mybir.dt.int16)
        nc.default_dma_engine.dma_start(out=idx_t[:], in_=idx_d.ap()[:, :])
        # per-(queue, slot) completion sems: a sem is locked to one SWDGE
        # queue, and per-slot rotation avoids aliasing a cumulative count
        # across skewed SDMA engines
        NSLOT = 3
        gsems = [[nc.alloc_semaphore(f"gather_sem{q}_{j}")
                  for j in range(NSLOT)] for q in range(NQUEUE)]

        ioff = 0   # idx column offset
        woff = 0   # w column offset
        r = 0      # global round index
        bi = 0     # batch index
        for g, ths in sig:
            base_px, nunits = PGROUPS[g]
            in_ap = bass.AP(tensor=table_d, offset=base_px * C,
                            ap=[[UNIT_ELEM, nunits], [1, UNIT_ELEM]])
            s_h = sum(ths)                 # half-tiles in batch
            s_c = (s_h + 1) // 2           # F-tile columns
            nidx = s_h * 64
            f_t = sbf.tile([128, s_c, UNIT_ELEM], mybir.dt.float16)
            qn = bi % NQUEUE
            sl = (bi // NQUEUE) % NSLOT
            nc.gpsimd.dma_gather(f_t[:], in_ap,
                                 idx_t[:, ioff:ioff + nidx // 16],
                                 nidx, nidx, UNIT_ELEM, elem_step=UNIT_ELEM,
                                 prepare_only=True, sem=gsems[qn][sl],
                                 queue_num=qn)
            nc.gpsimd.trigger_dma(count=None, queue_num=qn)
            nc.tensor.wait_ge(gsems[qn][sl],
                              16 * (bi // (NQUEUE * NSLOT) + 1))
            w_t = sbw.tile([128, s_c, UNIT_PX, NBIN], mybir.dt.float16)
            nc.default_dma_engine.dma_start(
                out=w_t[:].rearrange("p a b c -> p (a b c)"),
                in_=w_d.ap()[:, woff:woff + s_c * UNIT_PX * NBIN])
            n_r = len(ths)
            o_t = sbo.tile([NBIN, n_r, C], mybir.dt.bfloat16)
            hoff = 0
            for k, th in enumerate(ths):
                # K-slices covering half-tiles [hoff, hoff+th)
                slices = []                 # (col, poff, kk)
                h = hoff
                while h < hoff + th:
                    if h % 2 == 1:
                        slices.append((h // 2, 64, 64))
                        h += 1
                    elif hoff + th - h >= 2:
                        slices.append((h // 2, 0, 128))
                        h += 2
                    else:
                        slices.append((h // 2, 0, 64))
                        h += 1
                p_t = psm.tile([NBIN, C], mybir.dt.float32)
                n_s = len(slices)
                for si, (col, poff, kk) in enumerate(slices):
                    for q in range(UNIT_PX):
                        nc.tensor.matmul(
                            p_t[:],
                            lhsT=w_t[poff:poff + kk, col, q, :],
                            rhs=f_t[poff:poff + kk, col,
                                    q * C:(q + 1) * C],
                            start=(si == 0 and q == 0),
                            stop=(si == n_s - 1 and q == UNIT_PX - 1))
                nc.scalar.copy(out=o_t[:, k, :], in_=p_t[:])
                hoff += th
            out_ap = bass.AP(tensor=out_d, offset=r * NBIN * C,
                             ap=[[C, NBIN], [NBIN * C, n_r], [1, C]])
            nc.default_dma_engine.dma_start(out=out_ap, in_=o_t[:])
            r += n_r
            bi += 1
            ioff += nidx // 16
            woff += s_c * UNIT_PX * NBIN
    nc.compile()
    return nc


def _host_prep(f0, f1, f2, f3, boxes, bidx):
    boxes32 = np.asarray(boxes, np.float32)
    b = np.asarray(bidx).astype(np.int64)
    N = boxes32.shape[0]

    # level routing in strict fp32 (matches jax reference arithmetic)
    x1, y1, x2, y2 = (boxes32[:, k] for k in range(4))
    area = (x2 - x1 + np.float32(1.0)) * (y2 - y1 + np.float32(1.0))
    s = np.sqrt(area)
    lv = np.floor(np.float32(4.0) + np.log2(s / np.float32(224.0)
                                            + np.float32(1e-6)))
    lvl = (np.clip(lv, 2.0, 5.0)).astype(np.int64) - 2

    # channels-last flat fp16 table
    segs = []
    for f in (f0, f1, f2, f3):
        fa = np.asarray(f, np.float32)
        for bb in range(2):
            segs.append(np.transpose(fa[bb], (1, 2, 0)).reshape(-1, C)
                        .astype(np.float16))
    segs.append(np.zeros((END_PAD_PX, C), np.float16))
    table_flat = np.ascontiguousarray(np.concatenate(segs, 0)).reshape(-1)

    scs = np.array(SCALES)[lvl]
    Wl = np.array([hw[1] for hw in LVL_HW])[lvl]
    Hl = np.array([hw[0] for hw in LVL_HW])[lvl]
    x1s = boxes32[:, 0].astype(np.float64) * scs
    y1s = boxes32[:, 1].astype(np.float64) * scs
    x2s = boxes32[:, 2].astype(np.float64) * scs
    y2s = boxes32[:, 3].astype(np.float64) * scs
    bin_w = np.maximum(x2s - x1s, 1.0) / OUT
    bin_h = np.maximum(y2s - y1s, 1.0) / OUT
    grid = (np.arange(OUT)[:, None] + np.array([0.25, 0.75])[None, :]).reshape(-1)
    xs = x1s[:, None] + bin_w[:, None] * grid[None, :]     # [N,14]
    ys = y1s[:, None] + bin_h[:, None] * grid[None, :]
    vx = (xs >= -1.0) & (xs <= Wl[:, None])
    vy = (ys >= -1.0) & (ys <= Hl[:, None])
    xc = np.clip(xs, 0.0, (Wl - 1)[:, None])
    yc = np.clip(ys, 0.0, (Hl - 1)[:, None])
    x0c = np.minimum(np.floor(xc).astype(np.int64), (Wl - 2)[:, None])
    y0c = np.minimum(np.floor(yc).astype(np.int64), (Hl - 2)[:, None])
    lx = xc - x0c
    ly = yc - y0c

    seg_base = SEG_BASE[lvl, b]
    group = np.where(lvl == 0, b, np.where(lvl == 1, 2, 3))
    gbase = np.array([pg[0] for pg in PGROUPS])[group]

    yw = np.stack([1.0 - ly, ly], axis=2)                  # [N,14,2]
    xw = np.stack([1.0 - lx, lx], axis=2)                  # [N,14,2]
    # contribution grid [N, 14sy, 2t, 14sx, 2u]
    yrow = y0c[:, :, None] + np.arange(2)[None, None, :]   # [N,14,2]
    px_glob = (seg_base[:, None, None, None, None]
               + yrow[:, :, :, None, None] * Wl[:, None, None, None, None]
               + x0c[:, None, None, :, None]
               + np.arange(2)[None, None, None, None, :])
    rel = px_glob - gbase[:, None, None, None, None]
    unit = rel // UNIT_PX
    par = rel % UNIT_PX
    wgt = ((vy[:, :, None, None, None] & vx[:, None, None, :, None])
           * yw[:, :, :, None, None] * xw[:, None, None, :, :] * 0.25)
    sy_i = np.arange(14)
    binid = np.broadcast_to(
        ((sy_i // 2)[:, None, None, None] * 7 + (sy_i // 2)[None, None, :, None]),
        (14, 2, 14, 2)).ravel()

    # per-box dedup -> (group, units_u, Wbox[nslots,UNIT_PX,49], th, box_id)
    recs = []
    for n in range(N):
        pu, inv = np.unique(unit[n].ravel(), return_inverse=True)
        ns = len(pu)
        wb = np.zeros((ns, UNIT_PX, NBIN))
        np.add.at(wb, (inv, par[n].ravel(), binid), wgt[n].ravel())
        g = int(group[n])
        assert pu.min() >= 0 and pu.max() < PGROUPS[g][1]
        recs.append((g, pu.astype(np.int64), wb, 2 * ((ns + 127) // 128), n))

    # deal: per group, sort by unit count desc, pad to multiple of 8 with
    # dummies; round k takes sorted[8k:8k+8], core i gets 8k+i.
    rounds = []      # list of (group, th_round, [8 recs])
    for g in range(4):
        gr = [rc for rc in recs if rc[0] == g]
        gr.sort(key=lambda rc: -len(rc[1]))
        while len(gr) % 8:
            gr.append((g, np.zeros(1, np.int64),
                       np.zeros((1, UNIT_PX, NBIN)), 2, -1))
        for k in range(len(gr) // 8):
            eight = gr[8 * k:8 * k + 8]
            thr = max(rc[3] for rc in eight)
            rounds.append((g, thr, eight))

    # batches of consecutive same-group rounds, <= THBATCH half-tiles each
    batches = []     # (group, [round indices])
    for ri, (g, thr, _) in enumerate(rounds):
        if (batches and batches[-1][0] == g
                and sum(rounds[j][1] for j in batches[-1][1]) + thr <= THBATCH):
            batches[-1][1].append(ri)
        else:
            batches.append((g, [ri]))
    sig = tuple((g, tuple(rounds[j][1] for j in rjs)) for g, rjs in batches)

    # per-core streams, built per batch (rounds pack at half-tile offsets)
    idx_all, w_all = [], []
    omap = []        # per core: list over global rounds of box id (-1 dummy)
    for core in range(N_CORES):
        icols, wcols, cmap = [], [], []
        for g, rjs in batches:
            s_h = sum(rounds[j][1] for j in rjs)
            s_c = (s_h + 1) // 2
            ic = np.zeros(s_c * 128, np.int64)
            wp = np.zeros((s_c * 128, UNIT_PX, NBIN), np.float16)
            hoff = 0
            for j in rjs:
                _, thr, eight = rounds[j]
                _, pu, wb, _, box_id = eight[core]
                ns = len(pu)
                lo = hoff * 64
                ic[lo:lo + thr * 64] = pu[0]
                ic[lo:lo + ns] = pu
                wp[lo:lo + ns] = wb.astype(np.float16)
                cmap.append(box_id)
                hoff += thr
            assert ic.max() < 32768
            nidx = s_h * 64
            icols.append(np.tile(
                ic[:nidx].reshape(nidx // 16, 16).T.astype(np.int16), (8, 1)))
            wcols.append(wp.reshape(s_c, 128, UNIT_PX * NBIN)
                         .transpose(1, 0, 2)
                         .reshape(128, s_c * UNIT_PX * NBIN))
        idx_all.append(np.ascontiguousarray(np.concatenate(icols, axis=1)))
        w_all.append(np.ascontiguousarray(np.concatenate(wcols, axis=1)))
        omap.append(cmap)
    return table_flat, idx_all, w_all, omap, sig


LAST_RESULT = None


def kernel(f0, f1, f2, f3, boxes, box_batch_idx):
    global LAST_RESULT
    table_flat, idx_all, w_all, omap, sig = _host_prep(
        f0, f1, f2, f3, boxes, box_batch_idx)
    if sig not in _nc_cache:
        _nc_cache[sig] = _build_nc(sig)
    nc = _nc_cache[sig]
    in_maps = [{"table": table_flat, "idxs": idx_all[i], "wts": w_all[i]}
               for i in range(N_CORES)]
    res = bass_utils.run_bass_kernel_spmd(nc, in_maps,
                                          core_ids=list(range(N_CORES)))
    LAST_RESULT = res

    outfull = np.zeros((1024, NBIN, C), np.float32)
    for core in range(N_CORES):
        r = np.asarray(res.results[core]["out"]).astype(np.float32)
        for ri, box_id in enumerate(omap[core]):
            if box_id >= 0:
                outfull[box_id] = r[ri * NBIN:(ri + 1) * NBIN]
    return np.ascontiguousarray(
        outfull.transpose(0, 2, 1).reshape(1024, C, OUT, OUT))
